# revision 16
# baseline (speedup 1.0000x reference)
"""AttentionHead kernel for 8 TRN2 NeuronCores — key-split sharding.

Problem: q = x@Wq+bq; k = y@Wk+bk; v = y@Wv+bv
         att = softmax(q k^T / sqrt(128));  att = triu(att, k=1)  (AFTER softmax)
         out = att @ v
Shapes: x [4, 2048, 1024], y [4, 2048, 1024], W* [1024, 128], out [4, 2048, 128].

Sharding: 8 cores = (batch b in 0..3) x (key-parity par in 0..1).  Core
(b, par) computes ALL 2048 queries of batch b against the 8 key tiles
with global tile index g = 2t + par (t = local tile 0..7, 1024 local
keys).  Each core emits an UNNORMALIZED partial O^T [d, i] plus the
partial normalizer Z [1, i] (sum of exp over its local keys); the host
combines: out = ((O_even + O_odd) / (Z_even + Z_odd))^T.  No cross-core
communication, and softmax normalization (which spans ALL keys because
the triu mask is applied after softmax) is exact.

Why key-split beats query-split: k/v projections run over HALF the keys
(query-split duplicated them per core), and the post-softmax causal mask
drops half the P5 (att@v) work uniformly: interleaved key tiles give
every core, for every query chunk c, exactly 2 band tiles (local t=2c,
2c+1) and (6-2c) full tiles, with tiles t<2c fully masked and skipped.

Engine budget: the score exps can only run on the ACT engine and form
the kernel's serial co-bottleneck with the PE, so (a) adjacent score
tiles are computed into one 2-bank PSUM region and exponentiated as a
SINGLE [128,1024] activation (halves the per-op overhead; the two band
tiles of chunk c are tiles 2c/2c+1 = one pair, so the mask is one DVE
multiply per chunk), and (b) scores are emitted as a fine-grained
metronome — each as early as its supply allows, with projections/P5s/Z
threaded between pairs — so ACT never starves.  All other pointwise
work (biases, v-copies, ek pair-accumulation, finalize) rides the DVE;
GpSimd is useless for streaming ops (~1.2us/op dispatch overhead,
measured).

PSUM (8 banks): 2 rotating proj/Z banks + 2x2-bank score-pair region +
1 transpose bank + 1 O-accumulator bank.

DMA: one SP HWDGE ring sustains ~390GB/s (the per-core HBM fair share
of 8 concurrent cores); inputs are host-packed bf16 into one stream
tensor in exact consumption order, issued as 12 sequential dma_starts
whose boundaries are the consumer-visible semaphores.
"""

import numpy as np
import ml_dtypes

B = 4
LQ = 2048
LK = 2048
XS = 1024
PD = 128

NE = XS // 128  # 8 contraction tiles for projections
NTL = 8  # local key tiles per core (1024 keys)
NTP = NTL // 2  # 4 local tile PAIRS
CH = 512  # chunk (PSUM bank = 512 f32)
NCH = LQ // CH  # 4 query chunks
NKCL = 2  # local key chunks of 512
SM_SCALE = 1.0 / float(np.sqrt(PD))

WARM_N = 4  # warm-up matmuls (cold ~427ns each; bridge to chunk-1 landing)
FILL_MID = 2  # garbage matmuls between kproj e0-3 and e4-7
FILL_V = 1  # garbage matmuls before vproj
FILL_Q = 1  # garbage matmuls before qproj c0
FILL_G = 2  # garbage matmuls bridging the xc1 supply gap after scores c0

# stream column offsets (bf16 elements per partition).  Wq/xc0 ride
# BEFORE Wv: the first score pair needs kT+qT only, and vproj threads
# into the xc1 supply wait after the first scores.
OFF_WK = 0
OFF_Y0 = OFF_WK + XS  # y local tiles 0-3: 8e x 512
OFF_WQ = OFF_Y0 + NE * CH
OFF_X0 = OFF_WQ + XS  # x chunk c0
OFF_WV = OFF_X0 + NE * CH
OFF_TRI = OFF_WV + XS  # tri (mask graph only): p0 512 | p1 512


def _offs(apply_mask):
    tri_w = 1024 if apply_mask else 0
    off_x1 = OFF_TRI + tri_w
    off_y1 = off_x1 + NE * CH
    off_x2 = off_y1 + NE * CH
    off_x3 = off_x2 + NE * CH
    ncols = off_x3 + NE * CH
    return off_x1, off_y1, off_x2, off_x3, ncols


_BF16 = ml_dtypes.bfloat16

_graph_cache = {}


def _build_graph(apply_mask: bool):
    import concourse.mybir as mybir
    from concourse import bacc
    from concourse.masks import make_identity
    from concourse.tile import TileContext

    BF = mybir.dt.bfloat16
    F32 = mybir.dt.float32
    Exp = mybir.ActivationFunctionType.Exp

    OFF_X1, OFF_Y1, OFF_X2, OFF_X3, NCOLS = _offs(apply_mask)

    nc = bacc.Bacc()

    stream = nc.declare_dram_parameter("stream", [128, NCOLS], BF, isOutput=False)
    # cols 0..2 = bq, bk, bv
    consts = nc.declare_dram_parameter("consts", [128, 3], F32, isOutput=False)
    out_ext = nc.declare_dram_parameter("out", [PD, LQ], BF, isOutput=True)
    z_ext = nc.declare_dram_parameter("zout", [1, LQ], F32, isOutput=True)

    with TileContext(nc) as tc:
        with (
            tc.tile_pool(name="const", bufs=1) as const_pool,
            tc.tile_pool(name="sb", bufs=1) as sb_pool,
            tc.tile_pool(name="exp", bufs=3) as exp_pool,
            tc.tile_pool(name="ps", bufs=2, space="PSUM") as ps_pool,
            tc.tile_pool(name="psacc", bufs=1, space="PSUM") as psacc_pool,
        ):
            sin = sb_pool.tile([128, NCOLS], BF)
            consts_sb = const_pool.tile([128, 3], F32)

            # ---- input DMAs: one SP ring, strict consumption order.
            def sdma(lo, hi):
                nc.sync.dma_start(out=sin[:, lo:hi], in_=stream[:, lo:hi])

            sdma(OFF_WK, OFF_Y0 + 2 * CH)  # Wk + y-kc0 e0-1
            sdma(OFF_Y0 + 2 * CH, OFF_Y0 + 4 * CH)  # y-kc0 e2-3
            sdma(OFF_Y0 + 4 * CH, OFF_Y0 + 6 * CH)  # y-kc0 e4-5
            sdma(OFF_Y0 + 6 * CH, OFF_WQ + XS // 2)  # y-kc0 e6-7 + Wq e0-3
            sdma(OFF_WQ + XS // 2, OFF_X0 + 2 * CH)  # Wq e4-7 + x c0 e0-1
            sdma(OFF_X0 + 2 * CH, OFF_X0 + 5 * CH)  # x c0 e2-4
            sdma(OFF_X0 + 5 * CH, OFF_WV)  # x c0 e5-7
            sdma(OFF_WV, OFF_X1)  # Wv (+ tri in the mask graph)
            # back-half chunks split in e-halves: each projection starts
            # as soon as its first 4 e-steps land (~1us earlier), which
            # shrinks the exp-stream stall at every chunk boundary
            sdma(OFF_X1, OFF_X1 + 4 * CH)  # x c1 e0-3
            sdma(OFF_X1 + 4 * CH, OFF_Y1)  # x c1 e4-7
            sdma(OFF_Y1, OFF_Y1 + 4 * CH)  # y kc1 e0-3
            sdma(OFF_Y1 + 4 * CH, OFF_X2)  # y kc1 e4-7
            sdma(OFF_X2, OFF_X2 + 4 * CH)  # x c2 e0-3
            sdma(OFF_X2 + 4 * CH, OFF_X3)  # x c2 e4-7
            sdma(OFF_X3, OFF_X3 + 4 * CH)  # x c3 e0-3
            sdma(OFF_X3 + 4 * CH, NCOLS)  # x c3 e4-7

            def Wk_e(e):
                return sin[:, OFF_WK + e * 128:OFF_WK + (e + 1) * 128]

            def Wv_e(e):
                return sin[:, OFF_WV + e * 128:OFF_WV + (e + 1) * 128]

            def Wq_e(e):
                return sin[:, OFF_WQ + e * 128:OFF_WQ + (e + 1) * 128]

            def y_ap(kc, e):
                off = [OFF_Y0, OFF_Y1][kc] + e * CH
                return sin[:, off:off + CH]

            def x_ap(c, e):
                off = [OFF_X0, OFF_X1, OFF_X2, OFF_X3][c] + e * CH
                return sin[:, off:off + CH]

            tri_sb = sin[:, OFF_TRI:OFF_TRI + 1024] if apply_mask else None
            bq_sb = consts_sb[:, 0:1]
            bk_sb = consts_sb[:, 1:2]
            bv_sb = consts_sb[:, 2:3]

            # ---- constants, identity, ACT table prime on GpSimd.
            ones_sb = const_pool.tile([128, 128], BF)
            warm_rhs = const_pool.tile([128, CH], BF)
            nc.gpsimd.memset(warm_rhs, 1.0)
            nc.gpsimd.memset(ones_sb, 1.0)
            nc.gpsimd.dma_start(out=consts_sb, in_=consts[:, :])
            ident_sb = const_pool.tile([128, 128], BF)
            make_identity(nc, ident_sb)
            # Touch Exp early so the ~1.3us ACT_TABLE_LOAD overlaps the DMAs.
            scratch1 = const_pool.tile([1, 1], F32)
            nc.scalar.activation(scratch1, ones_sb[0:1, 0:1], Exp)
            warm_ps = ps_pool.tile(
                [128, CH], mybir.dt.float32, tag="rot", bufs=2, name="warm"
            )
            # transposes get a dedicated PSUM bank so score matmuls never
            # inherit WAR deps through the rotating pool
            pst_ps = psacc_pool.tile([128, NTL, PD], BF, tag="pst")

            def emit_warm(n):
                for _ in range(n):
                    nc.tensor.matmul(warm_ps, lhsT=ones_sb, rhs=warm_rhs,
                                     start=True, stop=True)

            emit_warm(WARM_N)

            kT_sb = sb_pool.tile([128, NKCL * CH], BF)
            vT_sb = sb_pool.tile([128, NKCL * CH], BF)
            v_sb = sb_pool.tile([128, NTL, PD], BF)
            qT_sb = sb_pool.tile([128, LQ], BF)
            z_ps = [None] * NCH

            def emit_qproj(c):
                cs = slice(c * CH, (c + 1) * CH)
                ps = ps_pool.tile(
                    [128, CH], mybir.dt.float32, tag="rot", bufs=2, name="qps"
                )
                for e in range(NE):
                    nc.tensor.matmul(
                        ps,
                        lhsT=Wq_e(e),
                        rhs=x_ap(c, e),
                        start=(e == 0),
                        stop=(e == NE - 1),
                    )
                nc.vector.tensor_scalar_add(qT_sb[:, cs], ps, bq_sb)

            def emit_ktproj(kc, mid_fill=0):
                ks = slice(kc * CH, (kc + 1) * CH)
                ps = ps_pool.tile(
                    [128, CH], mybir.dt.float32, tag="rot", bufs=2, name="kps"
                )
                for e in range(NE):
                    if e == 4 and mid_fill:
                        emit_warm(mid_fill)
                    nc.tensor.matmul(
                        ps,
                        lhsT=Wk_e(e),
                        rhs=y_ap(kc, e),
                        start=(e == 0),
                        stop=(e == NE - 1),
                    )
                nc.vector.tensor_scalar_add(kT_sb[:, ks], ps, bk_sb)

            def make_kt_fillers(kc):
                ks = slice(kc * CH, (kc + 1) * CH)
                ps = ps_pool.tile(
                    [128, CH], mybir.dt.float32, tag="rot", bufs=2, name="kps"
                )

                def step(e):
                    nc.tensor.matmul(
                        ps,
                        lhsT=Wk_e(e),
                        rhs=y_ap(kc, e),
                        start=(e == 0),
                        stop=(e == NE - 1),
                    )
                    if e == NE - 1:
                        nc.vector.tensor_scalar_add(kT_sb[:, ks], ps, bk_sb)

                return [lambda e=e: step(e) for e in range(NE)]

            deferred_vs = []

            def make_v_fillers(kc, defer_copies=False):
                ks = slice(kc * CH, (kc + 1) * CH)
                ps = ps_pool.tile(
                    [128, CH], mybir.dt.float32, tag="rot", bufs=2, name="vps"
                )

                def step(e):
                    nc.tensor.matmul(
                        ps,
                        lhsT=Wv_e(e),
                        rhs=y_ap(kc, e),
                        start=(e == 0),
                        stop=(e == NE - 1),
                    )
                    if e == NE - 1:
                        nc.vector.tensor_scalar_add(vT_sb[:, ks], ps, bv_sb)

                def trstep(t):
                    pst = pst_ps[:, t, :]
                    nc.tensor.transpose(
                        pst, vT_sb[:, t * 128:(t + 1) * 128], ident_sb
                    )
                    cp = lambda: nc.vector.tensor_copy(v_sb[:, t, :], pst)
                    deferred_vs.append(cp) if defer_copies else cp()

                return [lambda e=e: step(e) for e in range(NE)] + [
                    lambda t=t: trstep(t) for t in range(4 * kc, 4 * kc + 4)
                ]

            # ek pair-accumulators: [j, t_even's i | t_odd's i]
            ek_acc = [
                sb_pool.tile([128, 2 * CH], BF, name=f"ek_acc{c}")
                for c in range(NCH)
            ]
            e_tiles = {}
            deferred_ek = []
            ek_started = set()

            def emit_stp(tp, c, defer_ek=False):
                # score PAIR: local tiles 2tp, 2tp+1 vs query chunk c, one
                # [128,1024] exp.  Band pair for chunk c is tp == c.
                cs = slice(c * CH, (c + 1) * CH)
                stp = psacc_pool.tile(
                    [128, 2 * CH], mybir.dt.float32, tag="stp", bufs=2,
                    name="stp",
                )
                for h in (0, 1):
                    t = 2 * tp + h
                    nc.tensor.matmul(
                        stp[:, h * CH:(h + 1) * CH],
                        lhsT=kT_sb[:, t * 128:(t + 1) * 128],
                        rhs=qT_sb[:, cs],
                        start=True,
                        stop=True,
                    )
                e_sb = exp_pool.tile([128, 2 * CH], BF, bufs=8, name="e_sb")
                nc.scalar.activation(e_sb, stp, Exp, scale=SM_SCALE)
                e_tiles[2 * tp, c] = e_sb[:, 0:CH]
                e_tiles[2 * tp + 1, c] = e_sb[:, CH:2 * CH]

                # running unmasked pair-sum (Z = two ones-matmuls per chunk);
                # the in-place band mask multiplies afterwards.  Deferring
                # lets latency-critical DVE ops (vbias) jump ahead.
                def ek_op(tp=tp, c=c, e_sb=e_sb):
                    # first ek EMITTED for chunk c initializes the acc
                    # (chunk phases may start on the band pair, not tp 0)
                    if c not in ek_started:
                        ek_started.add(c)
                        nc.vector.tensor_copy(ek_acc[c], e_sb)
                    else:
                        nc.vector.tensor_add(ek_acc[c], ek_acc[c], e_sb)
                    if apply_mask and tp == c:
                        nc.vector.tensor_mul(e_sb, e_sb, tri_sb)

                deferred_ek.append(ek_op) if defer_ek else ek_op()

            o_ps = {}

            def emit_p5(t, c, start=False, stop=False):
                # O^T [d, i] += v_t^T @ maskedexp; tiles t<2c are all-zero
                # under the mask and never emitted.
                if c not in o_ps:
                    o_ps[c] = psacc_pool.tile(
                        [128, CH], mybir.dt.float32, tag="oacc", bufs=1,
                        name=f"o_ps{c}",
                    )
                w = 256 if (apply_mask and t == 2 * c and c < NCH - 1) else CH
                nc.tensor.matmul(
                    o_ps[c][:, 0:w],
                    lhsT=v_sb[:, t, :],
                    rhs=e_tiles[t, c][:, 0:w],
                    start=start,
                    stop=stop,
                )

            def p5_seq(c):
                # start on the full-width band (p1), stop on full-width t7
                # (c=3: the pair IS t6/t7, p0 kept full via tri zero-fill).
                if not apply_mask:
                    return [(t, t == 0, t == NTL - 1) for t in range(NTL)]
                if c == NCH - 1:
                    return [(7, True, False), (6, False, True)]
                mids = list(range(2 * c + 2, NTL - 1))
                return (
                    [(2 * c + 1, True, False)]
                    + [(t, False, False) for t in mids]
                    + [(2 * c, False, False), (NTL - 1, False, True)]
                )

            out_sb = sb_pool.tile([128, LQ], BF)
            z_sb = sb_pool.tile([1, LQ], F32)

            def emit_z(c):
                z_ps[c] = ps_pool.tile(
                    [128, CH], mybir.dt.float32, tag="rot", bufs=2, name=f"z{c}"
                )
                nc.tensor.matmul(
                    z_ps[c], lhsT=ones_sb, rhs=ek_acc[c][:, 0:CH],
                    start=True, stop=False,
                )
                nc.tensor.matmul(
                    z_ps[c], lhsT=ones_sb, rhs=ek_acc[c][:, CH:2 * CH],
                    start=False, stop=True,
                )

            def emit_finalize(c):
                cs = slice(c * CH, (c + 1) * CH)
                # unnormalized O^T + Z row out; host combines parities and
                # divides.  Copies on DVE (ACT stays a pure exp pipe).
                nc.vector.tensor_copy(out_sb[:, cs], o_ps[c])
                nc.vector.tensor_copy(z_sb[0:1, cs], z_ps[c][0:1, :])
                eng = nc.scalar if c % 2 == 0 else nc.sync
                eng.dma_start(out=out_ext[:, cs], in_=out_sb[:, cs])
                if c == NCH - 1:
                    nc.scalar.dma_start(out=z_ext[0:1, :], in_=z_sb[0:1, :])

            # ---- supply-ordered prologue: kproj then qproj (both supply-
            # paced), first score pair ASAP; vproj threads into the xc1
            # supply wait after the first scores.
            emit_ktproj(0, mid_fill=FILL_MID)
            emit_warm(FILL_Q)
            emit_qproj(0)
            emit_stp(0, 0, defer_ek=True)
            emit_stp(1, 0, defer_ek=True)
            fillers0 = make_v_fillers(0)  # vproj kc0 + tr + copies inline
            for f in fillers0:
                f()
            # eks/mask for the first two pairs ran deferred so vbias (and
            # the v-copies the P5s need) weren't queued behind them on DVE
            for op in deferred_ek:
                op()
            deferred_ek.clear()
            # qproj c1 + the c1 pairs go FIRST (the exp stream restarts as
            # soon as xc1 lands); c0's P5s then fill the y1 supply wait.
            emit_warm(FILL_G)
            emit_qproj(1)
            emit_stp(0, 1)
            emit_stp(1, 1)
            if apply_mask:
                emit_p5(1, 0, start=True)
                emit_p5(2, 0)
                emit_p5(3, 0)
                emit_p5(0, 0)  # shrunk p0 band (w=256)
            else:
                emit_p5(0, 0, start=True)
                emit_p5(1, 0)
                emit_p5(2, 0)
                emit_p5(3, 0)

            # kv kc1 (y1 lands during scores c1 t0-3); c0's t4-7 pairs ride
            # right behind kbias so ACT keeps streaming
            for f in make_kt_fillers(1):
                f()
            vff = make_v_fillers(1)
            for f in vff[:4]:  # vproj kc1 e0-3 (fills the kbias handoff)
                f()
            emit_stp(2, 0)
            for f in vff[4:8]:  # vproj kc1 e4-7
                f()
            emit_stp(3, 0)
            for f in vff[8:]:  # transposes t4-7 (+ copies)
                f()
            emit_stp(2, 1)
            emit_p5(4, 0)
            emit_p5(5, 0)
            emit_stp(3, 1)
            emit_qproj(2)  # qbias-c2 handoff hidden behind c0 P5s + Z
            emit_p5(6, 0)
            emit_p5(7, 0, stop=True)
            emit_z(0)
            emit_finalize(0)

            p51 = p5_seq(1)

            def pop_p5(queue, c, n):
                for _ in range(min(n, len(queue))):
                    t, st_, sp_ = queue.pop(0)
                    emit_p5(t, c, start=st_, stop=sp_)

            # c2 phase: band pair (2,2) FIRST so its mask clears long
            # before the c2 P5s; c1's P5 block threads between pairs
            emit_stp(2, 2)
            pop_p5(p51, 1, 2)
            emit_stp(3, 2)
            pop_p5(p51, 1, 2)
            emit_stp(0, 2)
            pop_p5(p51, 1, len(p51))
            emit_z(1)
            emit_finalize(1)
            emit_stp(1, 2)
            emit_qproj(3)
            # c3 phase: band pair (3,3) first for the same reason
            p52 = p5_seq(2)
            emit_stp(3, 3)
            pop_p5(p52, 2, 2)
            emit_stp(0, 3)
            pop_p5(p52, 2, len(p52))
            emit_z(2)
            emit_finalize(2)
            emit_stp(1, 3)
            emit_stp(2, 3)
            for t, st_, sp_ in p5_seq(3):
                emit_p5(t, 3, start=st_, stop=sp_)
            # tail: O copy + store immediately after the P5s (it does not
            # wait the Z chain); Z row follows.
            cs3 = slice(3 * CH, 4 * CH)
            nc.vector.tensor_copy(out_sb[:, cs3], o_ps[3])
            nc.sync.dma_start(out=out_ext[:, cs3], in_=out_sb[:, cs3])
            emit_z(3)
            nc.vector.tensor_copy(z_sb[0:1, cs3], z_ps[3][0:1, :])
            nc.scalar.dma_start(out=z_ext[0:1, :], in_=z_sb[0:1, :])

    nc.finalize()
    return nc


def _get_graph(apply_mask: bool):
    key = bool(apply_mask)
    if key not in _graph_cache:
        _graph_cache[key] = _build_graph(key)
    return _graph_cache[key]


def kernel(**inputs) -> np.ndarray:
    from concourse.bass_utils import run_bass_kernel_spmd

    x = np.asarray(inputs["x"], dtype=np.float32)
    y = np.asarray(inputs["y"], dtype=np.float32)
    Wq = np.asarray(inputs["Wq"], dtype=np.float32)
    Wk = np.asarray(inputs["Wk"], dtype=np.float32)
    Wv = np.asarray(inputs["Wv"], dtype=np.float32)
    bq = np.asarray(inputs["bq"], dtype=np.float32)
    bk = np.asarray(inputs["bk"], dtype=np.float32)
    bv = np.asarray(inputs["bv"], dtype=np.float32)
    mask = bool(np.asarray(inputs["mask"]).item())

    nc = _get_graph(mask)

    def pack_w(W):
        # [1024 f, 128 d] -> [128 p, 8 e, 128 d] -> [128, 1024]
        return W.reshape(NE, 128, PD).transpose(1, 0, 2).reshape(128, NE * PD)

    Wk_p, Wv_p, Wq_p = pack_w(Wk), pack_w(Wv), pack_w(Wq)

    if mask:
        jj = np.arange(128, dtype=np.int64)[:, None]
        ii = np.arange(512, dtype=np.int64)[None, :]
        tris = []
        for par in range(2):
            t0 = (jj + 128 * par > ii).astype(np.float32)  # p0 (live <= 256)
            t1 = (jj + 128 * (2 + par) > ii).astype(np.float32)  # p1
            tris.append(np.concatenate([t0, t1], axis=1))  # [128, 1024]

    consts_arr = np.ones((128, 3), dtype=np.float32)
    consts_arr[:, 0] = bq
    consts_arr[:, 1] = bk
    consts_arr[:, 2] = bv

    in_maps = []
    for core in range(8):
        b, par = core // 2, core % 2
        xs = x[b]  # [2048, 1024]
        ys = y[b].reshape(LK // 128, 128, XS)[par::2].reshape(NTL * 128, XS)
        # [seq, f] -> [8e, 128p, nch, 512] -> [128, nch, e*512+ii]
        xT4 = xs.T.reshape(NE, 128, NCH, CH).transpose(1, 2, 0, 3).reshape(
            128, NCH, NE * CH
        )
        yT4 = ys.T.reshape(NE, 128, NKCL, CH).transpose(1, 2, 0, 3).reshape(
            128, NKCL, NE * CH
        )
        parts = [Wk_p, yT4[:, 0], Wq_p, xT4[:, 0], Wv_p]
        if mask:
            parts.append(tris[par])
        parts += [xT4[:, 1], yT4[:, 1], xT4[:, 2], xT4[:, 3]]
        strm = np.ascontiguousarray(np.concatenate(parts, axis=1)).astype(_BF16)
        in_maps.append({"stream": strm, "consts": consts_arr})

    res = run_bass_kernel_spmd(nc, in_maps, core_ids=list(range(8)))

    out = np.empty((B, LQ, PD), dtype=np.float32)
    for b in range(B):
        oe = res.results[2 * b]["out"].astype(np.float32)  # [128 d, 2048 i]
        oo = res.results[2 * b + 1]["out"].astype(np.float32)
        ze = np.asarray(res.results[2 * b]["zout"], dtype=np.float32)
        zo = np.asarray(res.results[2 * b + 1]["zout"], dtype=np.float32)
        out[b] = ((oe + oo) / (ze + zo)).T
    return out


# revision 17
# speedup vs baseline: 1.0420x; 1.0420x over previous
"""AttentionHead kernel for 8 TRN2 NeuronCores — key-split sharding.

Problem: q = x@Wq+bq; k = y@Wk+bk; v = y@Wv+bv
         att = softmax(q k^T / sqrt(128));  att = triu(att, k=1)  (AFTER softmax)
         out = att @ v
Shapes: x [4, 2048, 1024], y [4, 2048, 1024], W* [1024, 128], out [4, 2048, 128].

Sharding: 8 cores = (batch b in 0..3) x (key-parity par in 0..1).  Core
(b, par) computes ALL 2048 queries of batch b against the 8 key tiles
with global tile index g = 2t + par (t = local tile 0..7, 1024 local
keys).  Each core emits an UNNORMALIZED partial O^T [d, i] plus the
partial normalizer Z [1, i] (sum of exp over its local keys); the host
combines: out = ((O_even + O_odd) / (Z_even + Z_odd))^T.  No cross-core
communication, and softmax normalization (which spans ALL keys because
the triu mask is applied after softmax) is exact.

Why key-split beats query-split: k/v projections run over HALF the keys
(query-split duplicated them per core), and the post-softmax causal mask
drops half the P5 (att@v) work uniformly: interleaved key tiles give
every core, for every query chunk c, exactly 2 band tiles (local t=2c,
2c+1) and (6-2c) full tiles, with tiles t<2c fully masked and skipped.

Engine budget: the score exps can only run on the ACT engine and form
the kernel's serial co-bottleneck with the PE, so (a) adjacent score
tiles are computed into one 2-bank PSUM region and exponentiated as a
SINGLE [128,1024] activation (halves the per-op overhead; the two band
tiles of chunk c are tiles 2c/2c+1 = one pair, so the mask is one DVE
multiply per chunk), and (b) scores are emitted as a fine-grained
metronome — each as early as its supply allows, with projections/P5s/Z
threaded between pairs — so ACT never starves.  All other pointwise
work (biases, v-copies, ek pair-accumulation, finalize) rides the DVE;
GpSimd is useless for streaming ops (~1.2us/op dispatch overhead,
measured).

PSUM (8 banks): 2 rotating proj/Z banks + 2x2-bank score-pair region +
1 transpose bank + 1 O-accumulator bank.

DMA: one SP HWDGE ring sustains ~390GB/s (the per-core HBM fair share
of 8 concurrent cores); inputs are host-packed bf16 into one stream
tensor in exact consumption order, issued as 12 sequential dma_starts
whose boundaries are the consumer-visible semaphores.
"""

import numpy as np
import ml_dtypes

B = 4
LQ = 2048
LK = 2048
XS = 1024
PD = 128

NE = XS // 128  # 8 contraction tiles for projections
NTL = 8  # local key tiles per core (1024 keys)
NTP = NTL // 2  # 4 local tile PAIRS
CH = 512  # chunk (PSUM bank = 512 f32)
NCH = LQ // CH  # 4 query chunks
NKCL = 2  # local key chunks of 512
SM_SCALE = 1.0 / float(np.sqrt(PD))

WARM_N = 4  # warm-up matmuls (cold ~427ns each; bridge to chunk-1 landing)
FILL_MID = 2  # garbage matmuls between kproj e0-3 and e4-7
FILL_V = 1  # garbage matmuls before vproj
FILL_Q = 1  # garbage matmuls before qproj c0
FILL_G = 2  # garbage matmuls bridging the xc1 supply gap after scores c0

# stream column offsets (bf16 elements per partition).  Wq/xc0 ride
# BEFORE Wv: the first score pair needs kT+qT only, and vproj threads
# into the xc1 supply wait after the first scores.
OFF_WK = 0
OFF_Y0 = OFF_WK + XS  # y local tiles 0-3: 8e x 512
OFF_WQ = OFF_Y0 + NE * CH
OFF_X0 = OFF_WQ + XS  # x chunk c0
OFF_WV = OFF_X0 + NE * CH
OFF_TRI = OFF_WV + XS  # tri (mask graph only): p0 512 | p1 512


def _offs(apply_mask):
    tri_w = 1024 if apply_mask else 0
    off_x1 = OFF_TRI + tri_w
    off_y1 = off_x1 + NE * CH
    off_x2 = off_y1 + NE * CH
    off_x3 = off_x2 + NE * CH
    ncols = off_x3 + NE * CH
    return off_x1, off_y1, off_x2, off_x3, ncols


_BF16 = ml_dtypes.bfloat16

_graph_cache = {}


def _build_graph(apply_mask: bool):
    import concourse.mybir as mybir
    from concourse import bacc
    from concourse.masks import make_identity
    from concourse.tile import TileContext

    BF = mybir.dt.bfloat16
    F32 = mybir.dt.float32
    Exp = mybir.ActivationFunctionType.Exp
    Ident = mybir.ActivationFunctionType.Identity

    OFF_X1, OFF_Y1, OFF_X2, OFF_X3, NCOLS = _offs(apply_mask)

    nc = bacc.Bacc()

    stream = nc.declare_dram_parameter("stream", [128, NCOLS], BF, isOutput=False)
    # cols 0..2 = bq, bk, bv
    consts = nc.declare_dram_parameter("consts", [128, 3], F32, isOutput=False)
    out_ext = nc.declare_dram_parameter("out", [PD, LQ], BF, isOutput=True)
    z_ext = nc.declare_dram_parameter("zout", [1, LQ], F32, isOutput=True)

    with TileContext(nc) as tc:
        with (
            tc.tile_pool(name="const", bufs=1) as const_pool,
            tc.tile_pool(name="sb", bufs=1) as sb_pool,
            tc.tile_pool(name="exp", bufs=3) as exp_pool,
            tc.tile_pool(name="ps", bufs=2, space="PSUM") as ps_pool,
            tc.tile_pool(name="psacc", bufs=1, space="PSUM") as psacc_pool,
        ):
            sin = sb_pool.tile([128, NCOLS], BF)
            consts_sb = const_pool.tile([128, 3], F32)

            # ---- input DMAs: one SP ring, strict consumption order.
            def sdma(lo, hi):
                nc.sync.dma_start(out=sin[:, lo:hi], in_=stream[:, lo:hi])

            sdma(OFF_WK, OFF_Y0 + 2 * CH)  # Wk + y-kc0 e0-1
            sdma(OFF_Y0 + 2 * CH, OFF_Y0 + 4 * CH)  # y-kc0 e2-3
            sdma(OFF_Y0 + 4 * CH, OFF_Y0 + 6 * CH)  # y-kc0 e4-5
            sdma(OFF_Y0 + 6 * CH, OFF_WQ + XS // 2)  # y-kc0 e6-7 + Wq e0-3
            sdma(OFF_WQ + XS // 2, OFF_X0 + 2 * CH)  # Wq e4-7 + x c0 e0-1
            sdma(OFF_X0 + 2 * CH, OFF_X0 + 5 * CH)  # x c0 e2-4
            sdma(OFF_X0 + 5 * CH, OFF_WV)  # x c0 e5-7
            sdma(OFF_WV, OFF_X1)  # Wv (+ tri in the mask graph)
            # back-half chunks split in e-halves: each projection starts
            # as soon as its first 4 e-steps land (~1us earlier), which
            # shrinks the exp-stream stall at every chunk boundary
            sdma(OFF_X1, OFF_X1 + 4 * CH)  # x c1 e0-3
            sdma(OFF_X1 + 4 * CH, OFF_Y1)  # x c1 e4-7
            sdma(OFF_Y1, OFF_Y1 + 4 * CH)  # y kc1 e0-3
            sdma(OFF_Y1 + 4 * CH, OFF_X2)  # y kc1 e4-7
            sdma(OFF_X2, OFF_X2 + 4 * CH)  # x c2 e0-3
            sdma(OFF_X2 + 4 * CH, OFF_X3)  # x c2 e4-7
            sdma(OFF_X3, OFF_X3 + 4 * CH)  # x c3 e0-3
            sdma(OFF_X3 + 4 * CH, NCOLS)  # x c3 e4-7

            def Wk_e(e):
                return sin[:, OFF_WK + e * 128:OFF_WK + (e + 1) * 128]

            def Wv_e(e):
                return sin[:, OFF_WV + e * 128:OFF_WV + (e + 1) * 128]

            def Wq_e(e):
                return sin[:, OFF_WQ + e * 128:OFF_WQ + (e + 1) * 128]

            def y_ap(kc, e):
                off = [OFF_Y0, OFF_Y1][kc] + e * CH
                return sin[:, off:off + CH]

            def x_ap(c, e):
                off = [OFF_X0, OFF_X1, OFF_X2, OFF_X3][c] + e * CH
                return sin[:, off:off + CH]

            tri_sb = sin[:, OFF_TRI:OFF_TRI + 1024] if apply_mask else None
            bq_sb = consts_sb[:, 0:1]
            bk_sb = consts_sb[:, 1:2]
            bv_sb = consts_sb[:, 2:3]

            # ---- constants, identity, ACT table prime on GpSimd.
            ones_sb = const_pool.tile([128, 128], BF)
            warm_rhs = const_pool.tile([128, CH], BF)
            nc.gpsimd.memset(warm_rhs, 1.0)
            nc.gpsimd.memset(ones_sb, 1.0)
            nc.gpsimd.dma_start(out=consts_sb, in_=consts[:, :])
            ident_sb = const_pool.tile([128, 128], BF)
            make_identity(nc, ident_sb)
            # Touch Exp early so the ~1.3us ACT_TABLE_LOAD overlaps the DMAs.
            scratch1 = const_pool.tile([1, 1], F32)
            nc.scalar.activation(scratch1, ones_sb[0:1, 0:1], Exp)
            warm_ps = ps_pool.tile(
                [128, CH], mybir.dt.float32, tag="rot", bufs=2, name="warm"
            )
            # transposes get a dedicated PSUM bank so score matmuls never
            # inherit WAR deps through the rotating pool
            pst_ps = psacc_pool.tile([128, NTL, PD], BF, tag="pst")

            def emit_warm(n):
                for _ in range(n):
                    nc.tensor.matmul(warm_ps, lhsT=ones_sb, rhs=warm_rhs,
                                     start=True, stop=True)

            emit_warm(WARM_N)

            kT_sb = sb_pool.tile([128, NKCL * CH], BF)
            vT_sb = sb_pool.tile([128, NKCL * CH], BF)
            v_sb = sb_pool.tile([128, NTL, PD], BF)
            qT_sb = sb_pool.tile([128, LQ], BF)
            z_ps = [None] * NCH

            def emit_qproj(c):
                cs = slice(c * CH, (c + 1) * CH)
                ps = ps_pool.tile(
                    [128, CH], mybir.dt.float32, tag="rot", bufs=2, name="qps"
                )
                for e in range(NE):
                    nc.tensor.matmul(
                        ps,
                        lhsT=Wq_e(e),
                        rhs=x_ap(c, e),
                        start=(e == 0),
                        stop=(e == NE - 1),
                    )
                nc.vector.tensor_scalar_add(qT_sb[:, cs], ps, bq_sb)

            def emit_ktproj(kc, mid_fill=0):
                ks = slice(kc * CH, (kc + 1) * CH)
                ps = ps_pool.tile(
                    [128, CH], mybir.dt.float32, tag="rot", bufs=2, name="kps"
                )
                for e in range(NE):
                    if e == 4 and mid_fill:
                        emit_warm(mid_fill)
                    nc.tensor.matmul(
                        ps,
                        lhsT=Wk_e(e),
                        rhs=y_ap(kc, e),
                        start=(e == 0),
                        stop=(e == NE - 1),
                    )
                nc.scalar.activation(kT_sb[:, ks], ps, Ident, bias=bk_sb)

            def make_kt_fillers(kc):
                ks = slice(kc * CH, (kc + 1) * CH)
                ps = ps_pool.tile(
                    [128, CH], mybir.dt.float32, tag="rot", bufs=2, name="kps"
                )

                def step(e):
                    nc.tensor.matmul(
                        ps,
                        lhsT=Wk_e(e),
                        rhs=y_ap(kc, e),
                        start=(e == 0),
                        stop=(e == NE - 1),
                    )
                    if e == NE - 1:
                        nc.scalar.activation(kT_sb[:, ks], ps, Ident, bias=bk_sb)

                return [lambda e=e: step(e) for e in range(NE)]

            deferred_vs = []

            def make_v_fillers(kc, defer_copies=False):
                ks = slice(kc * CH, (kc + 1) * CH)
                ps = ps_pool.tile(
                    [128, CH], mybir.dt.float32, tag="rot", bufs=2, name="vps"
                )

                def step(e):
                    nc.tensor.matmul(
                        ps,
                        lhsT=Wv_e(e),
                        rhs=y_ap(kc, e),
                        start=(e == 0),
                        stop=(e == NE - 1),
                    )
                    if e == NE - 1:
                        nc.scalar.activation(vT_sb[:, ks], ps, Ident, bias=bv_sb)

                def trstep(t):
                    pst = pst_ps[:, t, :]
                    nc.tensor.transpose(
                        pst, vT_sb[:, t * 128:(t + 1) * 128], ident_sb
                    )
                    cp = lambda: nc.vector.tensor_copy(v_sb[:, t, :], pst)
                    deferred_vs.append(cp) if defer_copies else cp()

                return [lambda e=e: step(e) for e in range(NE)] + [
                    lambda t=t: trstep(t) for t in range(4 * kc, 4 * kc + 4)
                ]

            # ek pair-accumulators: [j, t_even's i | t_odd's i]
            ek_acc = [
                sb_pool.tile([128, 2 * CH], BF, name=f"ek_acc{c}")
                for c in range(NCH)
            ]
            e_tiles = {}
            deferred_ek = []
            ek_started = set()

            def emit_stp(tp, c, defer_ek=False, skip_ek=False):
                # score PAIR: local tiles 2tp, 2tp+1 vs query chunk c, one
                # [128,1024] exp.  Band pair for chunk c is tp == c.
                cs = slice(c * CH, (c + 1) * CH)
                stp = psacc_pool.tile(
                    [128, 2 * CH], mybir.dt.float32, tag="stp", bufs=2,
                    name="stp",
                )
                for h in (0, 1):
                    t = 2 * tp + h
                    nc.tensor.matmul(
                        stp[:, h * CH:(h + 1) * CH],
                        lhsT=kT_sb[:, t * 128:(t + 1) * 128],
                        rhs=qT_sb[:, cs],
                        start=True,
                        stop=True,
                    )
                e_sb = exp_pool.tile([128, 2 * CH], BF, bufs=8, name="e_sb")
                nc.scalar.activation(e_sb, stp, Exp, scale=SM_SCALE)
                e_tiles[2 * tp, c] = e_sb[:, 0:CH]
                e_tiles[2 * tp + 1, c] = e_sb[:, CH:2 * CH]

                # running unmasked pair-sum (Z = two ones-matmuls per chunk);
                # the in-place band mask multiplies afterwards.  Deferring
                # lets latency-critical DVE ops (vbias) jump ahead.
                def ek_op(tp=tp, c=c, e_sb=e_sb):
                    # first ek EMITTED for chunk c initializes the acc
                    # (chunk phases may start on the band pair, not tp 0)
                    if c not in ek_started:
                        ek_started.add(c)
                        nc.vector.tensor_copy(ek_acc[c], e_sb)
                    else:
                        nc.vector.tensor_add(ek_acc[c], ek_acc[c], e_sb)
                    if apply_mask and tp == c:
                        nc.vector.tensor_mul(e_sb, e_sb, tri_sb)

                if not skip_ek:
                    deferred_ek.append(ek_op) if defer_ek else ek_op()

            o_ps = {}

            def emit_p5(t, c, start=False, stop=False):
                # O^T [d, i] += v_t^T @ maskedexp; tiles t<2c are all-zero
                # under the mask and never emitted.
                if c not in o_ps:
                    o_ps[c] = psacc_pool.tile(
                        [128, CH], mybir.dt.float32, tag="oacc", bufs=1,
                        name=f"o_ps{c}",
                    )
                w = 256 if (apply_mask and t == 2 * c and c < NCH - 1) else CH
                nc.tensor.matmul(
                    o_ps[c][:, 0:w],
                    lhsT=v_sb[:, t, :],
                    rhs=e_tiles[t, c][:, 0:w],
                    start=start,
                    stop=stop,
                )

            def p5_seq(c):
                # start on the full-width band (p1), stop on full-width t7
                # (c=3: the pair IS t6/t7, p0 kept full via tri zero-fill).
                if not apply_mask:
                    return [(t, t == 0, t == NTL - 1) for t in range(NTL)]
                if c == NCH - 1:
                    return [(7, True, False), (6, False, True)]
                mids = list(range(2 * c + 2, NTL - 1))
                return (
                    [(2 * c + 1, True, False)]
                    + [(t, False, False) for t in mids]
                    + [(2 * c, False, False), (NTL - 1, False, True)]
                )

            out_sb = sb_pool.tile([128, LQ], BF)
            z_sb = sb_pool.tile([1, LQ], F32)

            def emit_z(c, direct_pair=None):
                # direct_pair: a (tp, c) whose raw exp pair skipped the ek
                # accumulator and feeds Z straight from its e_sb halves --
                # removes the last DVE ek from the tail dependency chain.
                z_ps[c] = ps_pool.tile(
                    [128, CH], mybir.dt.float32, tag="rot", bufs=2, name=f"z{c}"
                )
                rhss = [ek_acc[c][:, 0:CH], ek_acc[c][:, CH:2 * CH]]
                if direct_pair is not None:
                    tp = direct_pair
                    rhss += [e_tiles[2 * tp, c], e_tiles[2 * tp + 1, c]]
                for i, rhs in enumerate(rhss):
                    nc.tensor.matmul(
                        z_ps[c], lhsT=ones_sb, rhs=rhs,
                        start=(i == 0), stop=(i == len(rhss) - 1),
                    )

            def emit_finalize(c):
                cs = slice(c * CH, (c + 1) * CH)
                # unnormalized O^T + Z row out; host combines parities and
                # divides.  Copies on DVE (ACT stays a pure exp pipe).
                nc.vector.tensor_copy(out_sb[:, cs], o_ps[c])
                nc.vector.tensor_copy(z_sb[0:1, cs], z_ps[c][0:1, :])
                eng = nc.scalar if c % 2 == 0 else nc.sync
                eng.dma_start(out=out_ext[:, cs], in_=out_sb[:, cs])
                if c == NCH - 1:
                    nc.scalar.dma_start(out=z_ext[0:1, :], in_=z_sb[0:1, :])

            # ---- supply-ordered prologue: kproj then qproj (both supply-
            # paced), first score pair ASAP; vproj threads into the xc1
            # supply wait after the first scores.
            emit_ktproj(0, mid_fill=FILL_MID)
            emit_warm(FILL_Q)
            emit_qproj(0)
            emit_stp(0, 0, defer_ek=True)
            emit_stp(1, 0, defer_ek=True)
            fillers0 = make_v_fillers(0)  # vproj kc0 + tr + copies inline
            for f in fillers0:
                f()
            # eks/mask for the first two pairs ran deferred so vbias (and
            # the v-copies the P5s need) weren't queued behind them on DVE
            for op in deferred_ek:
                op()
            deferred_ek.clear()
            # qproj c1 + the c1 pairs go FIRST (the exp stream restarts as
            # soon as xc1 lands); c0's P5s then fill the y1 supply wait.
            emit_warm(FILL_G)
            emit_qproj(1)
            emit_stp(0, 1)
            emit_stp(1, 1)
            if apply_mask:
                emit_p5(1, 0, start=True)
                emit_p5(2, 0)
                emit_p5(3, 0)
                emit_p5(0, 0)  # shrunk p0 band (w=256)
            else:
                emit_p5(0, 0, start=True)
                emit_p5(1, 0)
                emit_p5(2, 0)
                emit_p5(3, 0)

            # kv kc1 (y1 lands during scores c1 t0-3); c0's t4-7 pairs ride
            # right behind kbias so ACT keeps streaming
            for f in make_kt_fillers(1):
                f()
            vff = make_v_fillers(1)
            for f in vff[:4]:  # vproj kc1 e0-3 (fills the kbias handoff)
                f()
            emit_stp(2, 0)
            for f in vff[4:8]:  # vproj kc1 e4-7
                f()
            emit_stp(3, 0)
            for f in vff[8:]:  # transposes t4-7 (+ copies)
                f()
            emit_stp(2, 1)
            emit_p5(4, 0)
            emit_p5(5, 0)
            emit_stp(3, 1)
            emit_qproj(2)  # qbias-c2 handoff hidden behind c0 P5s + Z
            emit_p5(6, 0)
            emit_p5(7, 0, stop=True)
            emit_z(0)
            emit_finalize(0)

            p51 = p5_seq(1)

            def pop_p5(queue, c, n):
                for _ in range(min(n, len(queue))):
                    t, st_, sp_ = queue.pop(0)
                    emit_p5(t, c, start=st_, stop=sp_)

            # c2 phase: band pair (2,2) FIRST so its mask clears long
            # before the c2 P5s; c1's P5 block threads between pairs
            emit_stp(2, 2)
            pop_p5(p51, 1, 2)
            emit_stp(3, 2)
            pop_p5(p51, 1, 2)
            emit_stp(0, 2)
            pop_p5(p51, 1, len(p51))
            emit_z(1)
            emit_finalize(1)
            emit_stp(1, 2)
            emit_qproj(3)
            # c3 phase: band pair (3,3) first for the same reason
            p52 = p5_seq(2)
            emit_stp(3, 3)
            pop_p5(p52, 2, 2)
            emit_stp(0, 3)
            pop_p5(p52, 2, len(p52))
            emit_z(2)
            emit_finalize(2)
            emit_stp(1, 3)
            emit_stp(2, 3, skip_ek=True)  # Z-only pair, fed to Z directly
            for t, st_, sp_ in p5_seq(3):
                emit_p5(t, 3, start=st_, stop=sp_)
            # tail: O copy + store immediately after the P5s (it does not
            # wait the Z chain); Z row follows.
            cs3 = slice(3 * CH, 4 * CH)
            nc.vector.tensor_copy(out_sb[:, cs3], o_ps[3])
            nc.sync.dma_start(out=out_ext[:, cs3], in_=out_sb[:, cs3])
            emit_z(3, direct_pair=2)
            nc.vector.tensor_copy(z_sb[0:1, cs3], z_ps[3][0:1, :])
            nc.scalar.dma_start(out=z_ext[0:1, :], in_=z_sb[0:1, :])

    nc.finalize()
    return nc


def _get_graph(apply_mask: bool):
    key = bool(apply_mask)
    if key not in _graph_cache:
        _graph_cache[key] = _build_graph(key)
    return _graph_cache[key]


def kernel(**inputs) -> np.ndarray:
    from concourse.bass_utils import run_bass_kernel_spmd

    x = np.asarray(inputs["x"], dtype=np.float32)
    y = np.asarray(inputs["y"], dtype=np.float32)
    Wq = np.asarray(inputs["Wq"], dtype=np.float32)
    Wk = np.asarray(inputs["Wk"], dtype=np.float32)
    Wv = np.asarray(inputs["Wv"], dtype=np.float32)
    bq = np.asarray(inputs["bq"], dtype=np.float32)
    bk = np.asarray(inputs["bk"], dtype=np.float32)
    bv = np.asarray(inputs["bv"], dtype=np.float32)
    mask = bool(np.asarray(inputs["mask"]).item())

    nc = _get_graph(mask)

    def pack_w(W):
        # [1024 f, 128 d] -> [128 p, 8 e, 128 d] -> [128, 1024]
        return W.reshape(NE, 128, PD).transpose(1, 0, 2).reshape(128, NE * PD)

    Wk_p, Wv_p, Wq_p = pack_w(Wk), pack_w(Wv), pack_w(Wq)

    if mask:
        jj = np.arange(128, dtype=np.int64)[:, None]
        ii = np.arange(512, dtype=np.int64)[None, :]
        tris = []
        for par in range(2):
            t0 = (jj + 128 * par > ii).astype(np.float32)  # p0 (live <= 256)
            t1 = (jj + 128 * (2 + par) > ii).astype(np.float32)  # p1
            tris.append(np.concatenate([t0, t1], axis=1))  # [128, 1024]

    consts_arr = np.ones((128, 3), dtype=np.float32)
    consts_arr[:, 0] = bq
    consts_arr[:, 1] = bk
    consts_arr[:, 2] = bv

    in_maps = []
    for core in range(8):
        b, par = core // 2, core % 2
        xs = x[b]  # [2048, 1024]
        ys = y[b].reshape(LK // 128, 128, XS)[par::2].reshape(NTL * 128, XS)
        # [seq, f] -> [8e, 128p, nch, 512] -> [128, nch, e*512+ii]
        xT4 = xs.T.reshape(NE, 128, NCH, CH).transpose(1, 2, 0, 3).reshape(
            128, NCH, NE * CH
        )
        yT4 = ys.T.reshape(NE, 128, NKCL, CH).transpose(1, 2, 0, 3).reshape(
            128, NKCL, NE * CH
        )
        parts = [Wk_p, yT4[:, 0], Wq_p, xT4[:, 0], Wv_p]
        if mask:
            parts.append(tris[par])
        parts += [xT4[:, 1], yT4[:, 1], xT4[:, 2], xT4[:, 3]]
        strm = np.ascontiguousarray(np.concatenate(parts, axis=1)).astype(_BF16)
        in_maps.append({"stream": strm, "consts": consts_arr})

    res = run_bass_kernel_spmd(nc, in_maps, core_ids=list(range(8)))

    out = np.empty((B, LQ, PD), dtype=np.float32)
    for b in range(B):
        oe = res.results[2 * b]["out"].astype(np.float32)  # [128 d, 2048 i]
        oo = res.results[2 * b + 1]["out"].astype(np.float32)
        ze = np.asarray(res.results[2 * b]["zout"], dtype=np.float32)
        zo = np.asarray(res.results[2 * b + 1]["zout"], dtype=np.float32)
        out[b] = ((oe + oo) / (ze + zo)).T
    return out
